# revision 1
# baseline (speedup 1.0000x reference)
"""Multi-head attention + layernorm Bass kernel for Trainium2, 8 cores.

Problem: B=8, S=1024, D=768, H=12 heads x DH=64, key-padding mask, softmax,
output projection, layernorm.  Sharding: pure data parallelism — one batch
element per NeuronCore, no collectives.

Per-core layout (matmul operands fp16, accumulation fp32 in PSUM):
  - host passes x^T and all weights pre-arranged in SBUF layout so every
    load is one large-descriptor DMA; x on the sync queues, weights on the
    scalar-engine queues in parallel.
  - q^T/k^T built pair-of-heads-stacked: psum [128, 1024] = two heads' [64,S].
  - scores^T per key-chunk j: two concurrent row-tiled K=64 matmuls (heads
    at partitions 0-63 / 64-127); exp on ACT straight from PSUM with the
    key-padding mask folded into the per-partition bias.
  - ctx^T matmuls use V with a ones column appended, so softmax denominators
    fall out of psum row 64 of the same accumulation.
  - denominators: one batched DVE reciprocal per pair + one-hot-selector
    K=128 matmul broadcast + in-place multiply; chain runs at low priority
    so it never head-of-line blocks the PE queue.
  - out projection: pair-stacked K=128 chunks; +bo via rank-one matmul;
    layernorm via bn_stats/bn_aggr with scale/shift applied on ACT.
"""

import numpy as np

B, S, D, H, DH = 8, 1024, 768, 12, 64
NPAIR, NQUAD = H // 2, H // 4
SBLK = S // 128      # 8 key/row chunks
DCH = D // 128       # 6 contraction chunks
LN_EPS = 1e-5
NEG_MASK = -30.0

_PROGRAM = None


def _build_program():
    import concourse.bass as bass
    from concourse import bacc
    import concourse.tile as tile
    import concourse.mybir as mybir
    from contextlib import ExitStack

    F32 = mybir.dt.float32
    F32R = mybir.dt.float32r
    F16 = mybir.dt.float16
    AF = mybir.ActivationFunctionType

    nc = bacc.Bacc("TRN2", target_bir_lowering=False)

    xt_d = nc.dram_tensor("xt", [128, DCH * S], F16, kind="ExternalInput")
    wq_d = nc.dram_tensor("wq", [NPAIR, 128, DCH * 128], F16, kind="ExternalInput")
    wk_d = nc.dram_tensor("wk", [NPAIR, 128, DCH * 128], F16, kind="ExternalInput")
    wv_d = nc.dram_tensor("wv", [NQUAD, 128, DCH * 260], F16, kind="ExternalInput")
    wo_d = nc.dram_tensor("wo", [128, DCH * D], F16, kind="ExternalInput")
    bqk_d = nc.dram_tensor("bqk", [128, 2 * NPAIR], F32, kind="ExternalInput")
    bv_d = nc.dram_tensor("bv", [1, NQUAD * 260], F32, kind="ExternalInput")
    maskb_d = nc.dram_tensor("maskb", [128, SBLK], F32, kind="ExternalInput")
    gamma_d = nc.dram_tensor("gamma", [1, D], F32, kind="ExternalInput")
    beta_d = nc.dram_tensor("beta", [1, D], F32, kind="ExternalInput")
    sel_d = nc.dram_tensor("sel", [128, 256], F32R, kind="ExternalInput")
    onesr_d = nc.dram_tensor("onesr", [1, 128], F32R, kind="ExternalInput")
    bor_d = nc.dram_tensor("bor", [1, D], F32R, kind="ExternalInput")
    out_d = nc.dram_tensor("out", [S, D], F32, kind="ExternalOutput")

    with tile.TileContext(nc) as tc, ExitStack() as ctx:
        const = ctx.enter_context(tc.tile_pool(name="const", bufs=1))
        xt_p = ctx.enter_context(tc.tile_pool(name="xt_p", bufs=1))
        w_p = ctx.enter_context(tc.tile_pool(name="w_p", bufs=1))
        qk_p = ctx.enter_context(tc.tile_pool(name="qk_p", bufs=1))
        v_p = ctx.enter_context(tc.tile_pool(name="v_p", bufs=1))
        e_p = ctx.enter_context(tc.tile_pool(name="e_p", bufs=1))
        cx_p = ctx.enter_context(tc.tile_pool(name="cx_p", bufs=1))
        z_p = ctx.enter_context(tc.tile_pool(name="z_p", bufs=1))
        # 8 PSUM banks: proj 1x[128,1024] + scores 2x[128,1024] + cx 2x 1 bank
        ps = ctx.enter_context(tc.tile_pool(name="ps", bufs=1, space="PSUM"))

        # ---- x^T first: halves on sync + scalar queues in parallel ----
        xta0 = xt_p.tile([128, DCH // 2, S], F16, name="xta0")
        nc.sync.dma_start(out=xta0, in_=xt_d[:, :DCH // 2 * S])
        xta1 = xt_p.tile([128, DCH // 2, S], F16, name="xta1")
        nc.scalar.dma_start(out=xta1, in_=xt_d[:, DCH // 2 * S:])
        xt = [xta0[:, c, :] for c in range(DCH // 2)] +              [xta1[:, c, :] for c in range(DCH // 2)]

        # ---- small constants (sync queue, after x) ----
        bqk_t = const.tile([128, 2 * NPAIR], F32)
        nc.sync.dma_start(out=bqk_t, in_=bqk_d[:, :])
        bv_t = const.tile([128, NQUAD * 260], F32)
        nc.sync.dma_start(out=bv_t, in_=bv_d[0:1, :].to_broadcast([128, NQUAD * 260]))
        mask_t = const.tile([128, SBLK], F32)
        nc.sync.dma_start(out=mask_t, in_=maskb_d[:, :])
        sel_t = const.tile([128, 256], F32R)
        nc.sync.dma_start(out=sel_t, in_=sel_d[:, :])
        onesr_t = const.tile([1, 128], F32R)
        nc.sync.dma_start(out=onesr_t, in_=onesr_d[:, :])
        bor_t = const.tile([1, D], F32R)
        nc.sync.dma_start(out=bor_t, in_=bor_d[:, :])
        eps_t = const.tile([128, 1], F32)
        nc.vector.memset(eps_t, LN_EPS)

        wv_ts, wq_ts, wk_ts = [], [], []
        for q in range(NQUAD):
            wvq = w_p.tile([128, DCH, 260], F16, name="wvq", bufs=NQUAD)
            nc.scalar.dma_start(out=wvq, in_=wv_d[q])
            wv_ts.append([wvq[:, c, :] for c in range(DCH)])
        for p in range(NPAIR):
            wqp = w_p.tile([128, DCH, 128], F16, name="wqp", bufs=NPAIR)
            nc.scalar.dma_start(out=wqp, in_=wq_d[p])
            wq_ts.append([wqp[:, c, :] for c in range(DCH)])
            wkp = w_p.tile([128, DCH, 128], F16, name="wkp", bufs=NPAIR)
            nc.scalar.dma_start(out=wkp, in_=wk_d[p])
            wk_ts.append([wkp[:, c, :] for c in range(DCH)])

        # ---- v projections, per quad of heads ----
        v_sb = {}   # (quad, sblk) -> [128, 260] f16
        for q in range(NQUAD):
            wv_t = wv_ts[q]
            for s in range(SBLK):
                psv = ps.tile([128, 260], F32, name="psv",
                              tag=("proj" if s % 2 else "st"),
                              bufs=(1 if s % 2 else 2),
                              padded_shape=[128, 1024])
                for c in range(DCH):
                    nc.tensor.matmul(psv, xt[c][:, s * 128:(s + 1) * 128], wv_t[c],
                                     start=(c == 0), stop=(c == DCH - 1))
                vt = v_p.tile([128, 260], F16, name="v_sb", bufs=3 * SBLK)
                nc.vector.tensor_add(out=vt, in0=psv,
                                     in1=bv_t[:, q * 260:(q + 1) * 260])
                v_sb[(q, s)] = vt

        # ---- per pair: q/k projections then attention ----
        ctxt = []   # per pair [128, 1024] f16 normalized ctx^T (B at 64-127)

        for p in range(NPAIR):
            wq_t, wk_t = wq_ts[p], wk_ts[p]

            psq = ps.tile([128, S], F32, name="psq", tag="proj", bufs=1)
            for half in range(2):
                for c in range(DCH):
                    nc.tensor.matmul(
                        psq[:, half * 512:(half + 1) * 512], wq_t[c],
                        xt[c][:, half * 512:(half + 1) * 512],
                        start=(c == 0), stop=(c == DCH - 1))
            qt = qk_p.tile([128, S], F16, name="qt_sb", bufs=3)
            with tc.high_priority(offset=400):
                nc.vector.tensor_scalar_add(out=qt, in0=psq,
                                            scalar1=bqk_t[:, p:p + 1])

            psk = ps.tile([128, S], F32, name="psk", tag="proj", bufs=1)
            for half in range(2):
                for c in range(DCH):
                    nc.tensor.matmul(
                        psk[:, half * 512:(half + 1) * 512], wk_t[c],
                        xt[c][:, half * 512:(half + 1) * 512],
                        start=(c == 0), stop=(c == DCH - 1))
            kt = qk_p.tile([128, S], F16, name="kt_sb", bufs=3)
            with tc.high_priority(offset=400):
                nc.vector.tensor_scalar_add(
                    out=kt, in0=psk, scalar1=bqk_t[:, NPAIR + p:NPAIR + p + 1])

            ct = cx_p.tile([128, S], F16, name="ctxt", bufs=NPAIR)
            ctxt.append(ct)
            q, l0 = divmod(2 * p, 4)

            # denominators for this pair at partitions 32*(2*idx+iblk);
            # unused partitions start at 1.0 so the reciprocal stays finite
            rpk = z_p.tile([128, 512], F32, name="rpk", bufs=3)
            nc.vector.memset(rpk, 1.0)
            for iblk in range(2):
                pcx = [ps.tile([65, 512], F32, name="pscx", tag="cx", bufs=2)
                       for _ in range(2)]
                for j in range(SBLK):
                    pst = ps.tile([128, 1024], F32, name="psst", tag="st",
                                  bufs=2)
                    nc.tensor.matmul(
                        pst[:, 0:512], kt[0:64, j * 128:(j + 1) * 128],
                        qt[0:64, iblk * 512:(iblk + 1) * 512],
                        start=True, stop=True, tile_position=(0, 0))
                    nc.tensor.matmul(
                        pst[:, 512:1024], kt[64:128, j * 128:(j + 1) * 128],
                        qt[64:128, iblk * 512:(iblk + 1) * 512],
                        start=True, stop=True, tile_position=(64, 0))
                    et = e_p.tile([128, 1024], F16, name="expt", bufs=4)
                    nc.scalar.activation(et, pst, AF.Exp, bias=mask_t[:, j:j + 1])
                    for idx in range(2):
                        vsl = v_sb[(q, j)][:, (l0 + idx) * 65:(l0 + idx + 1) * 65]
                        nc.tensor.matmul(pcx[idx], vsl,
                                         et[:, idx * 512:(idx + 1) * 512],
                                         start=(j == 0), stop=(j == SBLK - 1))
                # move ctx (rows 0-63) and denominators (row 64) out of PSUM
                for idx in range(2):
                    u = 32 * (2 * idx + iblk)
                    nc.vector.tensor_copy(out=rpk[u:u + 1, :],
                                          in_=pcx[idx][64:65, :])
                    nc.vector.tensor_copy(
                        out=ct[idx * 64:(idx + 1) * 64,
                               iblk * 512:(iblk + 1) * 512],
                        in_=pcx[idx][0:64, :])
            # batched reciprocal + selector broadcast + in-place normalize;
            # only needed by the out-projection, so keep it off the PE
            # critical path
            rinv_p = z_p.tile([128, 512], F32R, name="rinv_p", bufs=3)
            with tc.high_priority(offset=-700):
                with nc.allow_low_precision(reason="f32r softmax denom"):
                    nc.vector.reciprocal(out=rinv_p, in_=rpk)
                for idx in range(2):
                    for iblk in range(2):
                        u = 2 * idx + iblk
                        pbc = ps.tile([64, 512], F32, name="psbc", tag="cx",
                                      bufs=2)
                        nc.tensor.matmul(pbc, sel_t[:, u * 64:(u + 1) * 64],
                                         rinv_p, start=True, stop=True)
                        csl = ct[idx * 64:(idx + 1) * 64,
                                 iblk * 512:(iblk + 1) * 512]
                        nc.vector.tensor_mul(out=csl, in0=csl, in1=pbc)

        # ---- output projection + layernorm, per row block ----
        gamma_t = const.tile([128, D], F32)
        nc.sync.dma_start(out=gamma_t, in_=gamma_d[0:1, :].to_broadcast([128, D]))
        beta_t = const.tile([128, D], F32)
        nc.sync.dma_start(out=beta_t, in_=beta_d[0:1, :].to_broadcast([128, D]))
        woa = w_p.tile([128, DCH, D], F16, name="woa", bufs=1)
        nc.scalar.dma_start(out=woa, in_=wo_d[:, :])
        wo_t = [woa[:, c, :] for c in range(DCH)]

        for s in range(SBLK):
            # alternate psum tags so out-proj blocks pipeline 3 deep
            pso = ps.tile([128, D], F32, name="pso",
                          tag=("proj" if s % 2 else "st"),
                          bufs=(1 if s % 2 else 2),
                          padded_shape=[128, 1024])
            for d0, d1 in ((0, 512), (512, 768)):
                for p in range(NPAIR):
                    nc.tensor.matmul(
                        pso[:, d0:d1],
                        ctxt[p][:, s * 128:(s + 1) * 128],
                        wo_t[p][:, d0:d1],
                        start=(p == 0), stop=False)
                # + bo via a K=1 rank-one update: ones_col x bo_row
                nc.tensor.matmul(pso[:, d0:d1], onesr_t, bor_t[:, d0:d1],
                                 start=False, stop=True)
            stats = z_p.tile([128, 3, 6], F32, name="stats", bufs=2)
            for g in range(3):
                nc.vector.bn_stats(out=stats[:, g, :],
                                   in_=pso[:, g * 256:(g + 1) * 256])
            mv = z_p.tile([128, 2], F32, name="mv", bufs=2)
            nc.vector.bn_aggr(out=mv, in_=stats)
            stdv = z_p.tile([128, 1], F32, name="stdv", bufs=2)
            nc.scalar.activation(stdv, mv[:, 1:2], AF.Sqrt, bias=eps_t)
            rstd = z_p.tile([128, 1], F32, name="rstd", bufs=2)
            nc.vector.reciprocal(out=rstd, in_=stdv)
            nmr = z_p.tile([128, 1], F32, name="nmr", bufs=2)
            nc.vector.tensor_scalar(out=nmr, in0=mv[:, 0:1], scalar1=rstd,
                                    scalar2=-1.0, op0=mybir.AluOpType.mult,
                                    op1=mybir.AluOpType.mult)
            z = z_p.tile([128, D], F32, name="z_sb", bufs=2)
            nc.scalar.activation(z, pso, AF.Identity, bias=nmr, scale=rstd)
            nc.vector.tensor_mul(out=z, in0=z, in1=gamma_t)
            nc.vector.tensor_add(out=z, in0=z, in1=beta_t)
            nc.sync.dma_start(out=out_d[s * 128:(s + 1) * 128, :], in_=z)

    nc.compile()
    return nc


def _host_inputs(inputs):
    x = np.asarray(inputs["input_tensor"], np.float32)
    mask = np.asarray(inputs["attention_mask"])
    Wq = np.asarray(inputs["Wq"], np.float32)
    bq = np.asarray(inputs["bq"], np.float32)
    Wk = np.asarray(inputs["Wk"], np.float32)
    bk = np.asarray(inputs["bk"], np.float32)
    Wv = np.asarray(inputs["Wv"], np.float32)
    bv = np.asarray(inputs["bv"], np.float32)
    Wo = np.asarray(inputs["Wo"], np.float32)
    bo = np.asarray(inputs["bo"], np.float32)
    gamma = np.asarray(inputs["gamma"], np.float32)
    beta = np.asarray(inputs["beta"], np.float32)

    scale = np.float32(1.0 / np.sqrt(DH))
    wq_flat = np.ascontiguousarray(
        (Wq * scale).transpose(1, 0, 2).reshape(D, D))
    wk_flat = np.ascontiguousarray(Wk.transpose(1, 0, 2).reshape(D, D))
    bq_s = (bq * scale).reshape(D)
    bk_s = bk.reshape(D)

    wv_aug = np.zeros((D, NQUAD * 260), np.float32)
    bv_aug = np.zeros((1, NQUAD * 260), np.float32)
    for h in range(H):
        q, l = divmod(h, 4)
        base = q * 260 + l * 65
        wv_aug[:, base:base + 64] = Wv[h]
        bv_aug[0, base:base + 64] = bv[h]
        bv_aug[0, base + 64] = 1.0

    bqk = np.zeros((128, 2 * NPAIR), np.float32)
    for p in range(NPAIR):
        bqk[:, p] = bq_s[p * 128:(p + 1) * 128]
        bqk[:, NPAIR + p] = bk_s[p * 128:(p + 1) * 128]

    sel = np.zeros((128, 256), np.float32)
    for u in range(4):
        sel[32 * u, u * 64:(u + 1) * 64] = 1.0

    def sbuf_layout(w, width):
        # [D, n*width] -> [n, 128, DCH*width]: partition-major per tile
        n = w.shape[1] // width
        return np.ascontiguousarray(
            w.reshape(DCH, 128, n, width).transpose(2, 1, 0, 3).reshape(
                n, 128, DCH * width).astype(np.float16))

    shared = {
        "wq": sbuf_layout(wq_flat, 128), "wk": sbuf_layout(wk_flat, 128),
        "wv": sbuf_layout(wv_aug, 260),
        "wo": sbuf_layout(np.ascontiguousarray(Wo), D)[0],
        "bqk": bqk, "bv": bv_aug,
        "gamma": gamma.reshape(1, D), "beta": beta.reshape(1, D),
        "sel": sel,
        "onesr": np.ones((1, 128), np.float32),
        "bor": bo.reshape(1, D).copy(),
    }
    in_maps = []
    for b in range(B):
        mb = np.where(mask[b], 0.0, NEG_MASK).astype(np.float32)
        in_maps.append({
            **shared,
            "xt": np.ascontiguousarray(
                x[b].T.reshape(DCH, 128, S).transpose(1, 0, 2).reshape(
                    128, DCH * S).astype(np.float16)),
            "maskb": np.ascontiguousarray(mb.reshape(SBLK, 128).T),
        })
    return in_maps


def _get_program():
    global _PROGRAM
    if _PROGRAM is None:
        _PROGRAM = _build_program()
    return _PROGRAM


def kernel(**inputs):
    from concourse.bass_utils import run_bass_kernel_spmd

    nc = _get_program()
    in_maps = _host_inputs(inputs)
    res = run_bass_kernel_spmd(nc, in_maps, list(range(B)))
    return np.stack([res.results[b]["out"] for b in range(B)], axis=0)


if __name__ == "__main__":
    rng = np.random.default_rng(0)
    demo = {
        "input_tensor": rng.standard_normal((B, S, D)).astype(np.float32),
        "attention_mask": np.ones((B, S), bool),
        "Wq": rng.standard_normal((H, D, DH)).astype(np.float32) * 0.03,
        "bq": rng.standard_normal((H, DH)).astype(np.float32) * 0.03,
        "Wk": rng.standard_normal((H, D, DH)).astype(np.float32) * 0.03,
        "bk": rng.standard_normal((H, DH)).astype(np.float32) * 0.03,
        "Wv": rng.standard_normal((H, D, DH)).astype(np.float32) * 0.03,
        "bv": rng.standard_normal((H, DH)).astype(np.float32) * 0.03,
        "Wo": rng.standard_normal((D, D)).astype(np.float32) * 0.03,
        "bo": rng.standard_normal((D,)).astype(np.float32) * 0.03,
        "gamma": np.ones((D,), np.float32),
        "beta": np.zeros((D,), np.float32),
    }
    out = kernel(**demo)
    print("kernel ran, out shape", out.shape, "finite:", np.isfinite(out).all())



# revision 16
# speedup vs baseline: 1.0881x; 1.0881x over previous
"""Multi-head attention + layernorm Bass kernel for Trainium2, 8 cores.

Problem: B=8, S=1024, D=768, H=12 heads x DH=64, key-padding mask, softmax,
output projection, layernorm.  Sharding: pure data parallelism - one batch
element per NeuronCore, no collectives.

v2 design (ACT-exp is the throughput floor at ~110us; everything else must
hide under it):
  - fp8e4(+DoubleRow, K=256/matmul) for q/k/v projections and attn@V; these
    errors enter before the softmax average and wash out.  Scores and the
    output projection stay fp16.
  - weights prescaled x64 on host so fp8 stays in the normal range; the
    scale cancels through the softmax normalize (ctx*64 times 1/(64*den)),
    and for scores it folds into the exp scale 2^-15.
  - iblk-outer attention; out-projection blocks s0..3 interleave under the
    second iblk so only s4..7 are an exposed tail.
  - softmax denominators: ones-column trick in V; reciprocal_approx_fast on
    the psum row; DMA row-broadcast; in-place f16 multiply.
  - ~130 warmup matmuls during the input DMA so HAM reaches K=8/8 before
    real work; xt split across 3 DMA queues.
"""

import numpy as np

B, S, D, H, DH = 8, 1024, 768, 12, 64
NPAIR, NQUAD = H // 2, H // 4
SBLK = S // 128      # 8 key/row chunks
DCH = D // 128       # 6 contraction chunks
LN_EPS = 1e-5
NEG_MASK = -30.0
W64 = 64.0           # host weight prescale
EXP_SCALE = 1.0 / (64.0 * 64.0 * 8.0)   # qt64*kt64 -> scores/8
VW = 65              # per-head stride in V layout: [v64, ones]
VQW = 4 * VW         # 260, per-quad width
FP8 = False
N_WARM = 130

_PROGRAM = None


def _build_program():
    import concourse.bass as bass
    from concourse import bacc
    import concourse.tile as tile
    import concourse.mybir as mybir
    from contextlib import ExitStack

    F32 = mybir.dt.float32
    F16 = mybir.dt.float16
    F8 = mybir.dt.float8e4
    FA = F8 if FP8 else F16
    DR = mybir.MatmulPerfMode.DoubleRow if FP8 else None
    AF = mybir.ActivationFunctionType
    CP = 2 if FP8 else 1          # contraction chunks consumed per matmul

    nc = bacc.Bacc("TRN2", target_bir_lowering=False)

    xt_d = nc.dram_tensor("xt", [128, DCH * S], FA, kind="ExternalInput")
    wq_d = nc.dram_tensor("wq", [NPAIR, 128, DCH * 128], FA, kind="ExternalInput")
    wk_d = nc.dram_tensor("wk", [NPAIR, 128, DCH * 128], FA, kind="ExternalInput")
    wv_d = nc.dram_tensor("wv", [NQUAD, 128, DCH * VQW], FA, kind="ExternalInput")
    wo_d = nc.dram_tensor("wo", [128, DCH * D], F16, kind="ExternalInput")
    bqk_d = nc.dram_tensor("bqk", [128, 2 * NPAIR], F32, kind="ExternalInput")
    bv_d = nc.dram_tensor("bv", [1, NQUAD * VQW], F32, kind="ExternalInput")
    maskb_d = nc.dram_tensor("maskb", [128, SBLK], F32, kind="ExternalInput")
    gamma_d = nc.dram_tensor("gamma", [1, D], F32, kind="ExternalInput")
    beta_d = nc.dram_tensor("beta", [1, D], F32, kind="ExternalInput")
    ones_d = nc.dram_tensor("ones16", [1, 128], F16, kind="ExternalInput")
    bo_d = nc.dram_tensor("bo16", [1, D], F16, kind="ExternalInput")
    out_d = nc.dram_tensor("out", [S, D], F32, kind="ExternalOutput")

    # j -> (et group, slot in group); groups pair key-chunks for DoubleRow
    ET_SLOT = [(0, 0), (0, 1), (3, 0), (1, 0), (1, 1), (3, 1), (2, 0), (2, 1)]
    # group -> (v dim1 slice start, stop, step)
    GRP_V = {0: (0, 2, 1), 1: (3, 5, 1), 2: (6, 8, 1), 3: (2, 6, 3)}

    with tile.TileContext(nc) as tc, ExitStack() as ctx:
        const = ctx.enter_context(tc.tile_pool(name="const", bufs=1))
        xt_p = ctx.enter_context(tc.tile_pool(name="xt_p", bufs=1))
        w_p = ctx.enter_context(tc.tile_pool(name="w_p", bufs=1))
        qk_p = ctx.enter_context(tc.tile_pool(name="qk_p", bufs=1))
        v_p = ctx.enter_context(tc.tile_pool(name="v_p", bufs=1))
        e_p = ctx.enter_context(tc.tile_pool(name="e_p", bufs=1))
        cx_p = ctx.enter_context(tc.tile_pool(name="cx_p", bufs=1))
        z_p = ctx.enter_context(tc.tile_pool(name="z_p", bufs=1))
        ps = ctx.enter_context(tc.tile_pool(name="ps", bufs=1, space="PSUM"))

        # ---- warmup stationary (DVE memset, no DMA dependency) ----
        warm16 = const.tile([128, 64], F16)
        nc.vector.memset(warm16, 0.25)

        # ---- input DMAs: xt on sync queue, chunk-pair granularity ----
        xt8 = xt_p.tile([128, DCH, S], FA, name="xt8")
        nc.sync.dma_start(out=xt8[:, 0:2, :], in_=xt_d[:, 0:2 * S])
        nc.sync.dma_start(out=xt8[:, 2:4, :], in_=xt_d[:, 2 * S:4 * S])
        nc.sync.dma_start(out=xt8[:, 4:6, :], in_=xt_d[:, 4 * S:6 * S])

        # small consts on sync queue
        bqk_t = const.tile([128, 2 * NPAIR], F32)
        nc.sync.dma_start(out=bqk_t, in_=bqk_d[:, :])
        mask_t = const.tile([128, SBLK], F32)
        nc.sync.dma_start(out=mask_t, in_=maskb_d[:, :])
        bv_t = const.tile([128, NQUAD * VQW], F32)
        nc.sync.dma_start(out=bv_t, in_=bv_d[0:1, :].to_broadcast([128, NQUAD * VQW]))
        ones_t = const.tile([1, 128], F16)
        nc.sync.dma_start(out=ones_t, in_=ones_d[:, :])
        bo_t = const.tile([1, D], F16)
        nc.sync.dma_start(out=bo_t, in_=bo_d[:, :])
        gamma_t = const.tile([128, D], F32)
        nc.sync.dma_start(out=gamma_t, in_=gamma_d[0:1, :].to_broadcast([128, D]))
        beta_t = const.tile([128, D], F32)
        nc.sync.dma_start(out=beta_t, in_=beta_d[0:1, :].to_broadcast([128, D]))
        eps_t = const.tile([128, 1], F32)
        nc.vector.memset(eps_t, LN_EPS)

        # weights: wq/wk interleaved per pair on scalar queue
        wq_ts, wk_ts = [], []
        for p in range(NPAIR):
            wqp = w_p.tile([128, DCH, 128], FA, name="wqp", bufs=NPAIR)
            nc.scalar.dma_start(out=wqp, in_=wq_d[p])
            wq_ts.append(wqp)
            wkp = w_p.tile([128, DCH, 128], FA, name="wkp", bufs=NPAIR)
            nc.scalar.dma_start(out=wkp, in_=wk_d[p])
            wk_ts.append(wkp)
        # wv on scalar queue after wq/wk
        wv_ts = []
        for q in range(NQUAD):
            wvq = w_p.tile([128, DCH, VQW], FA, name="wvq", bufs=NQUAD)
            nc.scalar.dma_start(out=wvq, in_=wv_d[q])
            wv_ts.append(wvq)
        # wo on gpsimd queue (needed only at the end)
        woa = w_p.tile([128, DCH, D], F16, name="woa", bufs=1)
        nc.gpsimd.dma_start(out=woa, in_=wo_d[:, :])

        # ---- PE warmup: keep HAM busy during input DMA ----
        pw = ps.tile([64, 64], F32, name="pw", tag="pa", bufs=2,
                     padded_shape=[128, 512])
        for _ in range(N_WARM):
            nc.tensor.matmul(pw, warm16, warm16, start=True, stop=True)

        # ---- emit helpers ----
        v8 = [v_p.tile([128, SBLK, VQW], FA, name="v8", bufs=NQUAD)
              for _ in range(NQUAD)]
        qt = [qk_p.tile([128, S], F16, name="qt", bufs=NPAIR) for _ in range(NPAIR)]
        kt = [qk_p.tile([128, S], F16, name="kt", bufs=NPAIR) for _ in range(NPAIR)]
        ct = [cx_p.tile([128, S], F16, name="ct", bufs=NPAIR) for _ in range(NPAIR)]

        def emit_vquad(q):
            wv_t = wv_ts[q]
            for s in range(SBLK):
                psv = ps.tile([128, 512], F32, name="psv", tag="pb", bufs=2,
                              padded_shape=[128, 1024])
                for ci in range(DCH // CP):
                    nc.tensor.matmul(
                        psv[:, 0:VQW],
                        xt8[:, CP * ci:CP * (ci + 1), s * 128:(s + 1) * 128],
                        wv_t[:, CP * ci:CP * (ci + 1), :],
                        start=(ci == 0), stop=(ci == DCH // CP - 1),
                        perf_mode=DR)
                with tc.high_priority(offset=350):
                    nc.vector.tensor_add(
                        out=v8[q][:, s, :], in0=psv[:, 0:VQW],
                        in1=bv_t[:, q * VQW:(q + 1) * VQW])

        def emit_proj(p):
            for dst, w_t, bcol in ((qt[p], wq_ts[p], p), (kt[p], wk_ts[p], NPAIR + p)):
                for half in range(2):
                    psq = ps.tile([128, 512], F32, name="psq", tag="pa", bufs=2)
                    for ci in range(DCH // CP):
                        nc.tensor.matmul(
                            psq,
                            w_t[:, CP * ci:CP * (ci + 1), :],
                            xt8[:, CP * ci:CP * (ci + 1),
                                half * 512:(half + 1) * 512],
                            start=(ci == 0), stop=(ci == DCH // CP - 1),
                            perf_mode=DR)
                    with tc.high_priority(offset=400):
                        nc.vector.tensor_scalar_add(
                            out=dst[:, half * 512:(half + 1) * 512], in0=psq,
                            scalar1=bqk_t[:, bcol:bcol + 1])

        def emit_attn(p, iblk):
            qx = 2 * p // 4          # quad holding this pair's heads
            l0 = (2 * p) % 4         # head offset within quad
            pcx = ps.tile([65, 1024], F32, name="pcx", tag="pc", bufs=1)
            ets = {}
            ngrp = 0
            for j in range(SBLK):
                pst = ps.tile([128, 1024], F32, name="pst", tag="pb", bufs=2)
                nc.tensor.matmul(
                    pst[:, 0:512], kt[p][0:64, j * 128:(j + 1) * 128],
                    qt[p][0:64, iblk * 512:(iblk + 1) * 512],
                    start=True, stop=True, tile_position=(0, 0))
                nc.tensor.matmul(
                    pst[:, 512:1024], kt[p][64:128, j * 128:(j + 1) * 128],
                    qt[p][64:128, iblk * 512:(iblk + 1) * 512],
                    start=True, stop=True, tile_position=(64, 0))
                g, t = ET_SLOT[j]
                if FP8:
                    if t == 0:
                        ets[g] = e_p.tile([128, 2, 1024], FA, name="et", bufs=4)
                    nc.scalar.activation(ets[g][:, t, :], pst, AF.Exp,
                                         bias=mask_t[:, j:j + 1],
                                         scale=EXP_SCALE)
                    if t == 1:
                        a, b, st = GRP_V[g]
                        for idx in range(2):
                            nc.tensor.matmul(
                                pcx[0:65, idx * 512:(idx + 1) * 512],
                                v8[qx][:, a:b:st,
                                       (l0 + idx) * VW:(l0 + idx + 1) * VW],
                                ets[g][:, :, idx * 512:(idx + 1) * 512],
                                start=(ngrp == 0), stop=(ngrp == 3),
                                perf_mode=DR)
                        ngrp += 1
                else:
                    et = e_p.tile([128, 1024], FA, name="et", bufs=4)
                    nc.scalar.activation(et, pst, AF.Exp,
                                         bias=mask_t[:, j:j + 1],
                                         scale=EXP_SCALE)
                    for idx in range(2):
                        nc.tensor.matmul(
                            pcx[0:65, idx * 512:(idx + 1) * 512],
                            v8[qx][:, j, (l0 + idx) * VW:(l0 + idx + 1) * VW],
                            et[:, idx * 512:(idx + 1) * 512],
                            start=(j == 0), stop=(j == SBLK - 1))
            # drain ctx (x64, rows 0..63) to f16; den row 64 via regular copy
            # (reciprocal_approx_fast ignores psum partition offsets)
            with tc.high_priority(offset=300):
                nc.vector.tensor_copy(
                    out=ct[p][0:64, iblk * 512:(iblk + 1) * 512],
                    in_=pcx[0:64, 0:512])
                nc.vector.tensor_copy(
                    out=ct[p][64:128, iblk * 512:(iblk + 1) * 512],
                    in_=pcx[0:64, 512:1024])
            rxs = z_p.tile([1, 1024], F32, name="rxs", bufs=3)
            rx = z_p.tile([1, 1024], F32, name="rx", bufs=3)
            pb32 = z_p.tile([128, 1024], F32, name="pb32", bufs=3)
            with tc.high_priority(offset=-500):
                nc.vector.tensor_copy(out=rxs, in_=pcx[64:65, 0:1024])
                nc.vector.reciprocal_approx_fast(out=rx, in_=rxs)
                nc.gpsimd.partition_broadcast(pb32, rx[0:1, :], channels=128)
                csl0 = ct[p][0:64, iblk * 512:(iblk + 1) * 512]
                nc.vector.tensor_mul(out=csl0, in0=csl0, in1=pb32[0:64, 0:512])
                csl1 = ct[p][64:128, iblk * 512:(iblk + 1) * 512]
                nc.vector.tensor_mul(out=csl1, in0=csl1, in1=pb32[64:128, 512:1024])

        def emit_out(s):
            pso = ps.tile([128, 1024], F32, name="pso", tag="pb", bufs=2)
            for d0, dn in ((0, 512), (512, 256)):
                for c in range(NPAIR):
                    nc.tensor.matmul(
                        pso[:, d0:d0 + dn],
                        ct[c][:, s * 128:(s + 1) * 128],
                        woa[:, c, d0:d0 + dn],
                        start=(c == 0), stop=False)
                nc.tensor.matmul(pso[:, d0:d0 + dn], ones_t,
                                 bo_t[0:1, d0:d0 + dn],
                                 start=False, stop=True)
            stats = z_p.tile([128, 3, 6], F32, name="stats", bufs=2)
            for g in range(3):
                nc.vector.bn_stats(out=stats[:, g, :],
                                   in_=pso[:, g * 256:(g + 1) * 256])
            mv = z_p.tile([128, 2], F32, name="mv", bufs=2)
            nc.vector.bn_aggr(out=mv, in_=stats)
            stdv = z_p.tile([128, 1], F32, name="stdv", bufs=2)
            nc.scalar.activation(stdv, mv[:, 1:2], AF.Sqrt, bias=eps_t)
            rstd = z_p.tile([128, 1], F32, name="rstd", bufs=2)
            nc.vector.reciprocal(out=rstd, in_=stdv)
            nmr = z_p.tile([128, 1], F32, name="nmr", bufs=2)
            nc.vector.tensor_scalar(out=nmr, in0=mv[:, 0:1], scalar1=rstd,
                                    scalar2=-1.0, op0=mybir.AluOpType.mult,
                                    op1=mybir.AluOpType.mult)
            z = z_p.tile([128, D], F32, name="z_sb", bufs=2)
            nc.scalar.activation(z[:, 0:512], pso[:, 0:512], AF.Identity,
                                 bias=nmr, scale=rstd)
            nc.scalar.activation(z[:, 512:768], pso[:, 512:768], AF.Identity,
                                 bias=nmr, scale=rstd)
            nc.gpsimd.tensor_mul(out=z, in0=z, in1=gamma_t)
            zf = z_p.tile([128, D], F32, name="zf", bufs=2)
            nc.vector.tensor_add(out=zf, in0=z, in1=beta_t)
            qd = nc.sync if s % 2 == 0 else nc.scalar
            qd.dma_start(out=out_d[s * 128:(s + 1) * 128, :], in_=zf)

        # ---- emission schedule ----
        emit_proj(0)
        emit_proj(1)
        emit_vquad(0)
        emit_attn(0, 0)
        emit_vquad(1)
        emit_proj(2)
        emit_attn(1, 0)
        emit_vquad(2)
        emit_proj(3)
        emit_attn(2, 0)
        emit_proj(4)
        emit_proj(5)
        emit_attn(3, 0)
        emit_attn(4, 0)
        emit_attn(5, 0)
        emit_attn(0, 1)
        emit_attn(1, 1)
        emit_out(0)
        emit_attn(2, 1)
        emit_out(1)
        emit_attn(3, 1)
        emit_out(2)
        emit_attn(4, 1)
        emit_out(3)
        emit_attn(5, 1)
        for s in range(4, SBLK):
            emit_out(s)

    nc.compile()
    return nc


def _np_f8():
    import ml_dtypes
    return ml_dtypes.float8_e4m3fn


def _host_inputs(inputs):
    x = np.asarray(inputs["input_tensor"], np.float32)
    mask = np.asarray(inputs["attention_mask"])
    Wq = np.asarray(inputs["Wq"], np.float32)
    bq = np.asarray(inputs["bq"], np.float32)
    Wk = np.asarray(inputs["Wk"], np.float32)
    bk = np.asarray(inputs["bk"], np.float32)
    Wv = np.asarray(inputs["Wv"], np.float32)
    bv = np.asarray(inputs["bv"], np.float32)
    Wo = np.asarray(inputs["Wo"], np.float32)
    bo = np.asarray(inputs["bo"], np.float32)
    gamma = np.asarray(inputs["gamma"], np.float32)
    beta = np.asarray(inputs["beta"], np.float32)

    fa = _np_f8() if FP8 else np.float16

    wq_flat = np.ascontiguousarray(Wq.transpose(1, 0, 2).reshape(D, D)) * W64
    wk_flat = np.ascontiguousarray(Wk.transpose(1, 0, 2).reshape(D, D)) * W64
    bq_s = bq.reshape(D) * W64
    bk_s = bk.reshape(D) * W64

    # ones column FIRST per head: denominator lands at psum partition 0
    wv_aug = np.zeros((D, NQUAD * VQW), np.float32)
    bv_aug = np.zeros((1, NQUAD * VQW), np.float32)
    for h in range(H):
        q, l = divmod(h, 4)
        base = q * VQW + l * VW
        wv_aug[:, base:base + 64] = Wv[h] * W64
        bv_aug[0, base:base + 64] = bv[h] * W64
        bv_aug[0, base + 64] = W64

    bqk = np.zeros((128, 2 * NPAIR), np.float32)
    for p in range(NPAIR):
        bqk[:, p] = bq_s[p * 128:(p + 1) * 128]
        bqk[:, NPAIR + p] = bk_s[p * 128:(p + 1) * 128]

    def sbuf_layout(w, width, dt):
        # [D, n*width] -> [n, 128, DCH*width]: partition-major per tile
        n = w.shape[1] // width
        return np.ascontiguousarray(
            w.reshape(DCH, 128, n, width).transpose(2, 1, 0, 3).reshape(
                n, 128, DCH * width).astype(dt))

    shared = {
        "wq": sbuf_layout(wq_flat, 128, fa),
        "wk": sbuf_layout(wk_flat, 128, fa),
        "wv": sbuf_layout(wv_aug, VQW, fa),
        "wo": sbuf_layout(np.ascontiguousarray(Wo), D, np.float16)[0],
        "bqk": bqk, "bv": bv_aug,
        "gamma": gamma.reshape(1, D).copy(),
        "beta": beta.reshape(1, D).copy(),
        "ones16": np.ones((1, 128), np.float16),
        "bo16": bo.reshape(1, D).astype(np.float16),
    }
    in_maps = []
    for b in range(B):
        mb = np.where(mask[b], 0.0, NEG_MASK).astype(np.float32)
        in_maps.append({
            **shared,
            "xt": np.ascontiguousarray(
                x[b].T.reshape(DCH, 128, S).transpose(1, 0, 2).reshape(
                    128, DCH * S).astype(fa)),
            "maskb": np.ascontiguousarray(mb.reshape(SBLK, 128).T),
        })
    return in_maps


def _get_program():
    global _PROGRAM
    if _PROGRAM is None:
        _PROGRAM = _build_program()
    return _PROGRAM


def kernel(**inputs):
    from concourse.bass_utils import run_bass_kernel_spmd

    nc = _get_program()
    in_maps = _host_inputs(inputs)
    res = run_bass_kernel_spmd(nc, in_maps, list(range(B)))
    return np.stack([res.results[b]["out"] for b in range(B)], axis=0)


if __name__ == "__main__":
    rng = np.random.default_rng(0)
    demo = {
        "input_tensor": rng.standard_normal((B, S, D)).astype(np.float32),
        "attention_mask": np.ones((B, S), bool),
        "Wq": (rng.standard_normal((H, D, DH)) * 0.03).astype(np.float32),
        "bq": (rng.standard_normal((H, DH)) * 0.03).astype(np.float32),
        "Wk": (rng.standard_normal((H, D, DH)) * 0.03).astype(np.float32),
        "bk": (rng.standard_normal((H, DH)) * 0.03).astype(np.float32),
        "Wv": (rng.standard_normal((H, D, DH)) * 0.03).astype(np.float32),
        "bv": (rng.standard_normal((H, DH)) * 0.03).astype(np.float32),
        "Wo": (rng.standard_normal((D, D)) * 0.03).astype(np.float32),
        "bo": (rng.standard_normal((D,)) * 0.03).astype(np.float32),
        "gamma": np.ones((D,), np.float32),
        "beta": np.zeros((D,), np.float32),
    }
    out = kernel(**demo)
    print("kernel ran, out shape", out.shape, "finite:", np.isfinite(out).all())


# revision 17
# speedup vs baseline: 1.1540x; 1.0605x over previous
"""Multi-head attention + layernorm Bass kernel for Trainium2, 8 cores.

Problem: B=8, S=1024, D=768, H=12 heads x DH=64, key-padding mask, softmax,
output projection, layernorm.  Sharding: pure data parallelism - one batch
element per NeuronCore, no collectives.

v2 design (ACT-exp is the throughput floor at ~110us; everything else must
hide under it):
  - fp8e4(+DoubleRow, K=256/matmul) for q/k/v projections and attn@V; these
    errors enter before the softmax average and wash out.  Scores and the
    output projection stay fp16.
  - weights prescaled x64 on host so fp8 stays in the normal range; the
    scale cancels through the softmax normalize (ctx*64 times 1/(64*den)),
    and for scores it folds into the exp scale 2^-15.
  - iblk-outer attention; out-projection blocks s0..3 interleave under the
    second iblk so only s4..7 are an exposed tail.
  - softmax denominators: ones-column trick in V; reciprocal_approx_fast on
    the psum row; DMA row-broadcast; in-place f16 multiply.
  - ~130 warmup matmuls during the input DMA so HAM reaches K=8/8 before
    real work; xt split across 3 DMA queues.
"""

import numpy as np

B, S, D, H, DH = 8, 1024, 768, 12, 64
NPAIR, NQUAD = H // 2, H // 4
SBLK = S // 128      # 8 key/row chunks
DCH = D // 128       # 6 contraction chunks
LN_EPS = 1e-5
NEG_MASK = -30.0
W64 = 64.0           # host weight prescale
EXP_SCALE = 1.0 / (64.0 * 64.0 * 8.0)   # qt64*kt64 -> scores/8
VW = 65              # per-head stride in V layout: [v64, ones]
VQW = 4 * VW         # 260, per-quad width
FP8 = False
N_WARM = 130

_PROGRAM = None


def _build_program():
    import concourse.bass as bass
    from concourse import bacc
    import concourse.tile as tile
    import concourse.mybir as mybir
    from contextlib import ExitStack

    F32 = mybir.dt.float32
    F16 = mybir.dt.float16
    F8 = mybir.dt.float8e4
    FA = F8 if FP8 else F16
    DR = mybir.MatmulPerfMode.DoubleRow if FP8 else None
    AF = mybir.ActivationFunctionType
    CP = 2 if FP8 else 1          # contraction chunks consumed per matmul

    nc = bacc.Bacc("TRN2", target_bir_lowering=False)

    xt_d = nc.dram_tensor("xt", [128, DCH * S], FA, kind="ExternalInput")
    wq_d = nc.dram_tensor("wq", [NPAIR, 128, DCH * 128], FA, kind="ExternalInput")
    wk_d = nc.dram_tensor("wk", [NPAIR, 128, DCH * 128], FA, kind="ExternalInput")
    wv_d = nc.dram_tensor("wv", [NQUAD, 128, DCH * VQW], FA, kind="ExternalInput")
    wo_d = nc.dram_tensor("wo", [128, DCH * D], F16, kind="ExternalInput")
    bqk_d = nc.dram_tensor("bqk", [128, 2 * NPAIR], F32, kind="ExternalInput")
    bv_d = nc.dram_tensor("bv", [1, NQUAD * VQW], F32, kind="ExternalInput")
    maskb_d = nc.dram_tensor("maskb", [128, SBLK], F32, kind="ExternalInput")
    gamma_d = nc.dram_tensor("gamma", [1, D], F32, kind="ExternalInput")
    beta_d = nc.dram_tensor("beta", [1, D], F32, kind="ExternalInput")
    ones_d = nc.dram_tensor("ones16", [1, 128], F16, kind="ExternalInput")
    bo_d = nc.dram_tensor("bo16", [1, D], F16, kind="ExternalInput")
    out_d = nc.dram_tensor("out", [S, D], F32, kind="ExternalOutput")

    # j -> (et group, slot in group); groups pair key-chunks for DoubleRow
    ET_SLOT = [(0, 0), (0, 1), (3, 0), (1, 0), (1, 1), (3, 1), (2, 0), (2, 1)]
    # group -> (v dim1 slice start, stop, step)
    GRP_V = {0: (0, 2, 1), 1: (3, 5, 1), 2: (6, 8, 1), 3: (2, 6, 3)}

    with tile.TileContext(nc) as tc, ExitStack() as ctx:
        const = ctx.enter_context(tc.tile_pool(name="const", bufs=1))
        xt_p = ctx.enter_context(tc.tile_pool(name="xt_p", bufs=1))
        w_p = ctx.enter_context(tc.tile_pool(name="w_p", bufs=1))
        qk_p = ctx.enter_context(tc.tile_pool(name="qk_p", bufs=1))
        v_p = ctx.enter_context(tc.tile_pool(name="v_p", bufs=1))
        e_p = ctx.enter_context(tc.tile_pool(name="e_p", bufs=1))
        cx_p = ctx.enter_context(tc.tile_pool(name="cx_p", bufs=1))
        z_p = ctx.enter_context(tc.tile_pool(name="z_p", bufs=1))
        ps = ctx.enter_context(tc.tile_pool(name="ps", bufs=1, space="PSUM"))

        # ---- warmup stationary (DVE memset, no DMA dependency) ----
        warm16 = const.tile([128, 64], F16)
        nc.vector.memset(warm16, 0.25)

        # ---- input DMAs: xt on sync queue, chunk-pair granularity ----
        xt8 = xt_p.tile([128, DCH, S], FA, name="xt8")
        nc.sync.dma_start(out=xt8[:, 0:2, :], in_=xt_d[:, 0:2 * S])
        nc.sync.dma_start(out=xt8[:, 2:4, :], in_=xt_d[:, 2 * S:4 * S])
        nc.gpsimd.dma_start(out=xt8[:, 4:6, :], in_=xt_d[:, 4 * S:6 * S])

        # small consts on sync queue
        bqk_t = const.tile([128, 2 * NPAIR], F32)
        nc.sync.dma_start(out=bqk_t, in_=bqk_d[:, :])
        mask_t = const.tile([128, SBLK], F32)
        nc.sync.dma_start(out=mask_t, in_=maskb_d[:, :])
        bv_t = const.tile([128, NQUAD * VQW], F32)
        nc.sync.dma_start(out=bv_t, in_=bv_d[0:1, :].to_broadcast([128, NQUAD * VQW]))
        ones_t = const.tile([1, 128], F16)
        nc.sync.dma_start(out=ones_t, in_=ones_d[:, :])
        bo_t = const.tile([1, D], F16)
        nc.sync.dma_start(out=bo_t, in_=bo_d[:, :])
        gamma_t = const.tile([128, D], F32)
        nc.sync.dma_start(out=gamma_t, in_=gamma_d[0:1, :].to_broadcast([128, D]))
        beta_t = const.tile([128, D], F32)
        nc.sync.dma_start(out=beta_t, in_=beta_d[0:1, :].to_broadcast([128, D]))
        eps_t = const.tile([128, 1], F32)
        nc.vector.memset(eps_t, LN_EPS)

        # weights: wq/wk interleaved per pair on scalar queue
        wq_ts, wk_ts = [], []
        for p in range(NPAIR):
            wqp = w_p.tile([128, DCH, 128], FA, name="wqp", bufs=NPAIR)
            nc.scalar.dma_start(out=wqp, in_=wq_d[p])
            wq_ts.append(wqp)
            wkp = w_p.tile([128, DCH, 128], FA, name="wkp", bufs=NPAIR)
            nc.scalar.dma_start(out=wkp, in_=wk_d[p])
            wk_ts.append(wkp)
        # wv on gpsimd queue (right after xt45, before wo)
        wv_ts = []
        for q in range(NQUAD):
            wvq = w_p.tile([128, DCH, VQW], FA, name="wvq", bufs=NQUAD)
            nc.gpsimd.dma_start(out=wvq, in_=wv_d[q])
            wv_ts.append(wvq)
        # wo on gpsimd queue (needed only at the end)
        woa = w_p.tile([128, DCH, D], F16, name="woa", bufs=1)
        nc.gpsimd.dma_start(out=woa, in_=wo_d[:, :])

        # ---- PE warmup: keep HAM busy during input DMA ----
        pw = ps.tile([64, 64], F32, name="pw", tag="pa", bufs=2,
                     padded_shape=[128, 512])
        for _ in range(N_WARM):
            nc.tensor.matmul(pw, warm16, warm16, start=True, stop=True)

        # ---- emit helpers ----
        v8 = [v_p.tile([128, SBLK, VQW], FA, name="v8", bufs=NQUAD)
              for _ in range(NQUAD)]
        qt = [qk_p.tile([128, S], F16, name="qt", bufs=NPAIR) for _ in range(NPAIR)]
        kt = [qk_p.tile([128, S], F16, name="kt", bufs=NPAIR) for _ in range(NPAIR)]
        ct = [cx_p.tile([128, S], F16, name="ct", bufs=NPAIR) for _ in range(NPAIR)]

        def emit_vquad(q):
            wv_t = wv_ts[q]
            for s in range(SBLK):
                psv = ps.tile([128, 512], F32, name="psv", tag="pb", bufs=2,
                              padded_shape=[128, 1024])
                for ci in range(DCH // CP):
                    nc.tensor.matmul(
                        psv[:, 0:VQW],
                        xt8[:, CP * ci:CP * (ci + 1), s * 128:(s + 1) * 128],
                        wv_t[:, CP * ci:CP * (ci + 1), :],
                        start=(ci == 0), stop=(ci == DCH // CP - 1),
                        perf_mode=DR)
                with tc.high_priority(offset=350):
                    nc.vector.tensor_add(
                        out=v8[q][:, s, :], in0=psv[:, 0:VQW],
                        in1=bv_t[:, q * VQW:(q + 1) * VQW])

        def emit_proj(p):
            for dst, w_t, bcol in ((qt[p], wq_ts[p], p), (kt[p], wk_ts[p], NPAIR + p)):
                for half in range(2):
                    psq = ps.tile([128, 512], F32, name="psq", tag="pa", bufs=2)
                    for ci in range(DCH // CP):
                        nc.tensor.matmul(
                            psq,
                            w_t[:, CP * ci:CP * (ci + 1), :],
                            xt8[:, CP * ci:CP * (ci + 1),
                                half * 512:(half + 1) * 512],
                            start=(ci == 0), stop=(ci == DCH // CP - 1),
                            perf_mode=DR)
                    with tc.high_priority(offset=400):
                        nc.vector.tensor_scalar_add(
                            out=dst[:, half * 512:(half + 1) * 512], in0=psq,
                            scalar1=bqk_t[:, bcol:bcol + 1])

        def emit_attn(p, iblk):
            qx = 2 * p // 4          # quad holding this pair's heads
            l0 = (2 * p) % 4         # head offset within quad
            pcx = ps.tile([65, 1024], F32, name="pcx", tag="pc", bufs=1)
            ets = {}
            ngrp = 0
            for j in range(SBLK):
                pst = ps.tile([128, 1024], F32, name="pst", tag="pb", bufs=2)
                nc.tensor.matmul(
                    pst[:, 0:512], kt[p][0:64, j * 128:(j + 1) * 128],
                    qt[p][0:64, iblk * 512:(iblk + 1) * 512],
                    start=True, stop=True, tile_position=(0, 0))
                nc.tensor.matmul(
                    pst[:, 512:1024], kt[p][64:128, j * 128:(j + 1) * 128],
                    qt[p][64:128, iblk * 512:(iblk + 1) * 512],
                    start=True, stop=True, tile_position=(64, 0))
                g, t = ET_SLOT[j]
                if FP8:
                    if t == 0:
                        ets[g] = e_p.tile([128, 2, 1024], FA, name="et", bufs=4)
                    nc.scalar.activation(ets[g][:, t, :], pst, AF.Exp,
                                         bias=mask_t[:, j:j + 1],
                                         scale=EXP_SCALE)
                    if t == 1:
                        a, b, st = GRP_V[g]
                        for idx in range(2):
                            nc.tensor.matmul(
                                pcx[0:65, idx * 512:(idx + 1) * 512],
                                v8[qx][:, a:b:st,
                                       (l0 + idx) * VW:(l0 + idx + 1) * VW],
                                ets[g][:, :, idx * 512:(idx + 1) * 512],
                                start=(ngrp == 0), stop=(ngrp == 3),
                                perf_mode=DR)
                        ngrp += 1
                else:
                    et = e_p.tile([128, 1024], FA, name="et", bufs=4)
                    nc.scalar.activation(et, pst, AF.Exp,
                                         bias=mask_t[:, j:j + 1],
                                         scale=EXP_SCALE)
                    for idx in range(2):
                        nc.tensor.matmul(
                            pcx[0:65, idx * 512:(idx + 1) * 512],
                            v8[qx][:, j, (l0 + idx) * VW:(l0 + idx + 1) * VW],
                            et[:, idx * 512:(idx + 1) * 512],
                            start=(j == 0), stop=(j == SBLK - 1))
            # drain ctx (x64, rows 0..63) to f16; den row 64 via regular copy
            # (reciprocal_approx_fast ignores psum partition offsets)
            with tc.high_priority(offset=300):
                nc.vector.tensor_copy(
                    out=ct[p][0:64, iblk * 512:(iblk + 1) * 512],
                    in_=pcx[0:64, 0:512])
                nc.vector.tensor_copy(
                    out=ct[p][64:128, iblk * 512:(iblk + 1) * 512],
                    in_=pcx[0:64, 512:1024])
            rxs = z_p.tile([1, 1024], F32, name="rxs", bufs=3)
            rx = z_p.tile([1, 1024], F32, name="rx", bufs=3)
            pb32 = z_p.tile([128, 1024], F32, name="pb32", bufs=3)
            with tc.high_priority(offset=-500):
                nc.vector.tensor_copy(out=rxs, in_=pcx[64:65, 0:1024])
                nc.vector.reciprocal_approx_fast(out=rx, in_=rxs)
                nc.gpsimd.partition_broadcast(pb32, rx[0:1, :], channels=128)
                csl0 = ct[p][0:64, iblk * 512:(iblk + 1) * 512]
                nc.vector.tensor_mul(out=csl0, in0=csl0, in1=pb32[0:64, 0:512])
                csl1 = ct[p][64:128, iblk * 512:(iblk + 1) * 512]
                nc.vector.tensor_mul(out=csl1, in0=csl1, in1=pb32[64:128, 512:1024])

        def emit_out(s):
            # pso in the pa ring: free during attention, so scores never
            # stall on the LN drain chain
            pso_a = ps.tile([128, 512], F32, name="pso_a", tag="pa", bufs=2)
            pso_b = ps.tile([128, 512], F32, name="pso_b", tag="pa", bufs=2)
            for pt, d0, dn in ((pso_a, 0, 512), (pso_b, 512, 256)):
                for c in range(NPAIR):
                    nc.tensor.matmul(
                        pt[:, 0:dn],
                        ct[c][:, s * 128:(s + 1) * 128],
                        woa[:, c, d0:d0 + dn],
                        start=(c == 0), stop=False)
                nc.tensor.matmul(pt[:, 0:dn], ones_t,
                                 bo_t[0:1, d0:d0 + dn],
                                 start=False, stop=True)
            stats = z_p.tile([128, 3, 6], F32, name="stats", bufs=2)
            with tc.high_priority(offset=600):
                nc.vector.bn_stats(out=stats[:, 0, :], in_=pso_a[:, 0:256])
                nc.vector.bn_stats(out=stats[:, 1, :], in_=pso_a[:, 256:512])
                nc.vector.bn_stats(out=stats[:, 2, :], in_=pso_b[:, 0:256])
                mv = z_p.tile([128, 2], F32, name="mv", bufs=2)
                nc.vector.bn_aggr(out=mv, in_=stats)
            stdv = z_p.tile([128, 1], F32, name="stdv", bufs=2)
            with tc.high_priority(offset=2000):
                nc.scalar.activation(stdv, mv[:, 1:2], AF.Sqrt, bias=eps_t)
            with tc.high_priority(offset=600):
                rstd = z_p.tile([128, 1], F32, name="rstd", bufs=2)
                nc.vector.reciprocal(out=rstd, in_=stdv)
                nmr = z_p.tile([128, 1], F32, name="nmr", bufs=2)
                nc.vector.tensor_scalar(out=nmr, in0=mv[:, 0:1], scalar1=rstd,
                                        scalar2=-1.0, op0=mybir.AluOpType.mult,
                                        op1=mybir.AluOpType.mult)
                z = z_p.tile([128, D], F32, name="z_sb", bufs=2)
                nc.vector.tensor_scalar(out=z[:, 0:512], in0=pso_a,
                                        scalar1=rstd, scalar2=nmr,
                                        op0=mybir.AluOpType.mult,
                                        op1=mybir.AluOpType.add)
                nc.vector.tensor_scalar(out=z[:, 512:768], in0=pso_b[:, 0:256],
                                        scalar1=rstd, scalar2=nmr,
                                        op0=mybir.AluOpType.mult,
                                        op1=mybir.AluOpType.add)
            nc.gpsimd.tensor_mul(out=z, in0=z, in1=gamma_t)
            zf = z_p.tile([128, D], F32, name="zf", bufs=2)
            nc.vector.tensor_add(out=zf, in0=z, in1=beta_t)
            qd = nc.sync if s % 2 == 0 else nc.gpsimd
            qd.dma_start(out=out_d[s * 128:(s + 1) * 128, :], in_=zf)

        # ---- emission schedule ----
        emit_proj(0)
        emit_proj(1)
        emit_vquad(0)
        emit_attn(0, 0)
        emit_vquad(1)
        emit_proj(2)
        emit_attn(1, 0)
        emit_vquad(2)
        emit_proj(3)
        emit_attn(2, 0)
        emit_proj(4)
        emit_proj(5)
        emit_attn(3, 0)
        emit_attn(4, 0)
        emit_attn(5, 0)
        emit_attn(0, 1)
        emit_attn(1, 1)
        emit_out(0)
        emit_attn(2, 1)
        emit_out(1)
        emit_attn(3, 1)
        emit_out(2)
        emit_attn(4, 1)
        emit_out(3)
        emit_attn(5, 1)
        for s in range(4, SBLK):
            emit_out(s)

    nc.compile()
    return nc


def _np_f8():
    import ml_dtypes
    return ml_dtypes.float8_e4m3fn


def _host_inputs(inputs):
    x = np.asarray(inputs["input_tensor"], np.float32)
    mask = np.asarray(inputs["attention_mask"])
    Wq = np.asarray(inputs["Wq"], np.float32)
    bq = np.asarray(inputs["bq"], np.float32)
    Wk = np.asarray(inputs["Wk"], np.float32)
    bk = np.asarray(inputs["bk"], np.float32)
    Wv = np.asarray(inputs["Wv"], np.float32)
    bv = np.asarray(inputs["bv"], np.float32)
    Wo = np.asarray(inputs["Wo"], np.float32)
    bo = np.asarray(inputs["bo"], np.float32)
    gamma = np.asarray(inputs["gamma"], np.float32)
    beta = np.asarray(inputs["beta"], np.float32)

    fa = _np_f8() if FP8 else np.float16

    wq_flat = np.ascontiguousarray(Wq.transpose(1, 0, 2).reshape(D, D)) * W64
    wk_flat = np.ascontiguousarray(Wk.transpose(1, 0, 2).reshape(D, D)) * W64
    bq_s = bq.reshape(D) * W64
    bk_s = bk.reshape(D) * W64

    # ones column FIRST per head: denominator lands at psum partition 0
    wv_aug = np.zeros((D, NQUAD * VQW), np.float32)
    bv_aug = np.zeros((1, NQUAD * VQW), np.float32)
    for h in range(H):
        q, l = divmod(h, 4)
        base = q * VQW + l * VW
        wv_aug[:, base:base + 64] = Wv[h] * W64
        bv_aug[0, base:base + 64] = bv[h] * W64
        bv_aug[0, base + 64] = W64

    bqk = np.zeros((128, 2 * NPAIR), np.float32)
    for p in range(NPAIR):
        bqk[:, p] = bq_s[p * 128:(p + 1) * 128]
        bqk[:, NPAIR + p] = bk_s[p * 128:(p + 1) * 128]

    def sbuf_layout(w, width, dt):
        # [D, n*width] -> [n, 128, DCH*width]: partition-major per tile
        n = w.shape[1] // width
        return np.ascontiguousarray(
            w.reshape(DCH, 128, n, width).transpose(2, 1, 0, 3).reshape(
                n, 128, DCH * width).astype(dt))

    shared = {
        "wq": sbuf_layout(wq_flat, 128, fa),
        "wk": sbuf_layout(wk_flat, 128, fa),
        "wv": sbuf_layout(wv_aug, VQW, fa),
        "wo": sbuf_layout(np.ascontiguousarray(Wo), D, np.float16)[0],
        "bqk": bqk, "bv": bv_aug,
        "gamma": gamma.reshape(1, D).copy(),
        "beta": beta.reshape(1, D).copy(),
        "ones16": np.ones((1, 128), np.float16),
        "bo16": bo.reshape(1, D).astype(np.float16),
    }
    in_maps = []
    for b in range(B):
        mb = np.where(mask[b], 0.0, NEG_MASK).astype(np.float32)
        in_maps.append({
            **shared,
            "xt": np.ascontiguousarray(
                x[b].T.reshape(DCH, 128, S).transpose(1, 0, 2).reshape(
                    128, DCH * S).astype(fa)),
            "maskb": np.ascontiguousarray(mb.reshape(SBLK, 128).T),
        })
    return in_maps


def _get_program():
    global _PROGRAM
    if _PROGRAM is None:
        _PROGRAM = _build_program()
    return _PROGRAM


def kernel(**inputs):
    from concourse.bass_utils import run_bass_kernel_spmd

    nc = _get_program()
    in_maps = _host_inputs(inputs)
    res = run_bass_kernel_spmd(nc, in_maps, list(range(B)))
    return np.stack([res.results[b]["out"] for b in range(B)], axis=0)


if __name__ == "__main__":
    rng = np.random.default_rng(0)
    demo = {
        "input_tensor": rng.standard_normal((B, S, D)).astype(np.float32),
        "attention_mask": np.ones((B, S), bool),
        "Wq": (rng.standard_normal((H, D, DH)) * 0.03).astype(np.float32),
        "bq": (rng.standard_normal((H, DH)) * 0.03).astype(np.float32),
        "Wk": (rng.standard_normal((H, D, DH)) * 0.03).astype(np.float32),
        "bk": (rng.standard_normal((H, DH)) * 0.03).astype(np.float32),
        "Wv": (rng.standard_normal((H, D, DH)) * 0.03).astype(np.float32),
        "bv": (rng.standard_normal((H, DH)) * 0.03).astype(np.float32),
        "Wo": (rng.standard_normal((D, D)) * 0.03).astype(np.float32),
        "bo": (rng.standard_normal((D,)) * 0.03).astype(np.float32),
        "gamma": np.ones((D,), np.float32),
        "beta": np.zeros((D,), np.float32),
    }
    out = kernel(**demo)
    print("kernel ran, out shape", out.shape, "finite:", np.isfinite(out).all())
